# revision 5
# baseline (speedup 1.0000x reference)
"""GateLoop (B=4, N=4096, D=1024) Trainium2 kernel over 8 NeuronCores.

Sharding: data-parallel over the 4 batch elements x 2-way tensor-parallel
split of the D=1024 recurrence channels (the complex diagonal recurrence is
independent per channel). Core c handles batch c//2, channels
[(c%2)*512 : (c%2+1)*512]. Each core computes its projections, runs the
scan over the full sequence for its 512 channels, and produces a partial
y @ wo[ch, :] of shape (1024, 4096) (transposed). The host sums the two
partials per batch and transposes back. No cross-core communication.

Scan formulation (avoids complex arithmetic + overflow): with
a_t = m_t * cis(phi_t), m_t = sigmoid(|a_t|), write theta_t =
arctan(ai/ar) in (-pi/2, pi/2) and fold the ar<0 half-plane flip into a
SIGNED real multiplier mt_t = m_t * sign(ar_t). With U_t = cis(Theta_t),
Theta_t = cumsum(theta), and S_t = U_t * Z_t the recurrence
S_t = a_t S_{t-1} + kv_t becomes two independent REAL first-order scans
    Zr_t = mt_t * Zr_{t-1} + kv_t * cos(Theta_t)
    Zi_t = mt_t * Zi_{t-1} + kv_t * sin(Theta_t)
and Re(S_t) = cos(Theta_t) * Zr_t + sin(Theta_t) * Zi_t, which map onto
the DVE TensorTensorScan instruction (fp32 state, |mt| < 1 so stable).

v3 performance structure:
 - The output projection of block b is emitted AFTER the projections of
   block b+1 so the PE never waits on the elementwise chain: it streams
   proj(b+1) then out(b) back-to-back and stays at the 2.4 GHz p-state.
 - war/wai PSUM banks are evacuated with plain copies first, derived math
   reads the SBUF copies (banks freed in ~0.7us instead of ~2.5us).
 - Elementwise work is split: GpSimd takes off-critical-path fp32 ops
   (squares, r2, ratio, wr, t1, magic-rounding); DVE keeps the scan
   backbone, PSUM-reading ops, and the short output tail; ACT keeps the
   table functions grouped (sqrt | sigmoid+arctan | sin+silu) to bound
   activation-table loads.
"""
import math
import os

import numpy as np
import ml_dtypes

B, N, D = 4, 4096, 1024
CH = 512            # channels per core (tensor-parallel half)
NCG = CH // 128     # 4 channel groups of 128 partitions
T = 512             # token block
NBLK = N // T
P = 128
KT = D // P         # contraction tiles
EPS = 1e-5
BF16 = ml_dtypes.bfloat16

TWO_PI = 2 * math.pi
C1 = float(np.float32(6.28125))
C2 = float(np.float32(np.float64(TWO_PI) - 6.28125))
C3 = float(np.float32(np.float64(TWO_PI) - 6.28125
                      - np.float64(np.float32(np.float64(TWO_PI) - 6.28125))))
MAGIC = float(np.float32(1.5 * 2 ** 23))
INV2PI = float(np.float32(1.0 / TWO_PI))
PI = float(np.float32(math.pi))
PIH = float(np.float32(math.pi / 2))

_NC = None
LAST_RESULT = None  # BassKernelResults of the most recent run (for profiling)


def _build():
    from contextlib import ExitStack
    from concourse import bacc
    import concourse.mybir as mybir
    import concourse.tile as tile
    from concourse.mybir import ActivationFunctionType as AF, AluOpType as OP

    fp32 = mybir.dt.float32
    bf = mybir.dt.bfloat16

    nc = bacc.Bacc(None, target_bir_lowering=False)

    xnT_d = nc.dram_tensor("xnT", [D, N], bf, kind="ExternalInput")
    wnames = ["wq", "wk", "wv", "wg", "war", "wai"]
    w_d = {n: nc.dram_tensor(n, [D, CH], bf, kind="ExternalInput") for n in wnames}
    wo_d = nc.dram_tensor("wo", [CH, D], bf, kind="ExternalInput")
    outT_d = nc.dram_tensor("outT", [D, N], fp32, kind="ExternalOutput")

    xnT_t = xnT_d.rearrange("(ko p) n -> p ko n", p=P)
    outT_t = outT_d.rearrange("(mo p) n -> p mo n", p=P)

    with tile.TileContext(nc) as tc, ExitStack() as ctx:
        wpool = ctx.enter_context(tc.tile_pool(name="w", bufs=1))
        xpool = ctx.enter_context(tc.tile_pool(name="x", bufs=2))
        cpool = ctx.enter_context(tc.tile_pool(name="c", bufs=1))   # per-cg, within-block
        kpool = ctx.enter_context(tc.tile_pool(name="k", bufs=1))   # scan outputs (carry via *_C)
        carry = ctx.enter_context(tc.tile_pool(name="cc", bufs=2))  # [P,1] scan carries
        scr = ctx.enter_context(tc.tile_pool(name="s", bufs=10))    # short-lived scratch
        obp = ctx.enter_context(tc.tile_pool(name="o", bufs=2))
        ypool = ctx.enter_context(tc.tile_pool(name="y", bufs=2))
        pproj = ctx.enter_context(tc.tile_pool(name="pp", bufs=6, space="PSUM"))
        pout = ctx.enter_context(tc.tile_pool(name="po", bufs=2, space="PSUM"))

        wsb = {}
        for n in wnames:
            t_ = wpool.tile([P, KT, CH], bf, tag=f"w_{n}")
            nc.sync.dma_start(t_[:], w_d[n].rearrange("(ko p) m -> p ko m", p=P))
            wsb[n] = t_
        wosb = wpool.tile([P, CH // P, D], bf, tag="w_wo")
        nc.sync.dma_start(wosb[:], wo_d.rearrange("(ko p) m -> p ko m", p=P))

        prevThC = [None] * NCG
        prevZrC = [None] * NCG
        prevZiC = [None] * NCG
        pending_out = None  # (blk, ys[4]) whose output matmul is deferred

        def emit_out(blk, ys):
            t0 = blk * T
            for mo in range(D // P):
                pso = pout.tile([P, T], fp32, tag="out")
                for cg in range(NCG):
                    nc.tensor.matmul(pso[:], wosb[:, cg, mo * P:(mo + 1) * P],
                                     ys[cg][:], start=(cg == 0), stop=(cg == NCG - 1))
                ob = obp.tile([P, T], fp32, tag="ob")
                nc.scalar.copy(ob[:], pso[:])
                nc.sync.dma_start(outT_t[:, mo, t0:t0 + T], ob[:])

        for blk in range(NBLK):
            t0 = blk * T
            xb = xpool.tile([P, KT, T], bf, tag="xb")
            nc.sync.dma_start(xb[:], xnT_t[:, :, t0:t0 + T])

            kv = [None] * NCG; qs = [None] * NCG; gs = [None] * NCG
            sgn = [None] * NCG; ratio = [None] * NCG; r2 = [None] * NCG
            for cg in range(NCG):
                cs = slice(cg * P, (cg + 1) * P)
                PS = {}
                for n in wnames:
                    ps = pproj.tile([P, T], fp32, tag="proj")
                    for k in range(KT):
                        nc.tensor.matmul(ps[:], wsb[n][:, k, cs], xb[:, k, :],
                                         start=(k == 0), stop=(k == KT - 1))
                    PS[n] = ps
                    # evacuate each PSUM bank as soon as its matmuls finish
                    if n == "wq":
                        qs[cg] = cpool.tile([P, T], bf, tag=f"qs{cg}", name=f"qs{cg}_{blk}")
                        nc.scalar.copy(qs[cg][:], ps[:])
                    elif n == "wk":
                        pass  # read by kv right after wv's copy lands
                    elif n == "wv":
                        vs = scr.tile([P, T], fp32, tag="scr", name=f"vs{cg}_{blk}")
                        nc.scalar.copy(vs[:], ps[:])
                        kv[cg] = cpool.tile([P, T], fp32, tag=f"kv{cg}", name=f"kv{cg}_{blk}")
                        nc.vector.tensor_tensor(kv[cg][:], PS["wk"][:], vs[:], OP.mult)
                    elif n == "wg":
                        gs[cg] = cpool.tile([P, T], bf, tag=f"gs{cg}", name=f"gs{cg}_{blk}")
                        nc.scalar.copy(gs[cg][:], ps[:])
                    elif n == "war":
                        war_s = scr.tile([P, T], fp32, tag="scr", name=f"war{cg}_{blk}")
                        nc.scalar.copy(war_s[:], ps[:])
                        PS["war_s"] = war_s
                    elif n == "wai":
                        wai_s = scr.tile([P, T], fp32, tag="scr", name=f"wai{cg}_{blk}")
                        nc.scalar.copy(wai_s[:], ps[:])
                        PS["wai_s"] = wai_s
                # derived P1 math from the SBUF copies
                war_s, wai_s = PS["war_s"], PS["wai_s"]
                sq = scr.tile([P, T], fp32, tag="scr")
                nc.gpsimd.tensor_tensor(sq[:], war_s[:], war_s[:], OP.mult)
                sq2 = scr.tile([P, T], fp32, tag="scr")
                nc.gpsimd.tensor_tensor(sq2[:], wai_s[:], wai_s[:], OP.mult)
                r2[cg] = cpool.tile([P, T], fp32, tag=f"r2{cg}", name=f"r2{cg}_{blk}")
                nc.gpsimd.tensor_tensor(r2[cg][:], sq[:], sq2[:], OP.add)
                sgn[cg] = cpool.tile([P, T], bf, tag=f"sgn{cg}", name=f"sgn{cg}_{blk}")
                nc.scalar.sign(sgn[cg][:], war_s[:])
                aabs = scr.tile([P, T], fp32, tag="scr")
                nc.scalar.activation(aabs[:], war_s[:], AF.Abs)
                nc.vector.tensor_scalar(aabs[:], aabs[:], 1e-4, None, OP.max)
                rec = scr.tile([P, T], fp32, tag="scr")
                nc.vector.reciprocal_approx_fast(rec[:], aabs[:])
                ratio[cg] = cpool.tile([P, T], fp32, tag=f"ratio{cg}", name=f"ratio{cg}_{blk}")
                nc.gpsimd.tensor_tensor(ratio[cg][:], wai_s[:], rec[:], OP.mult)

            # deferred output projection of the PREVIOUS block: the PE flows
            # straight from this block's projections into it, no stall.
            if pending_out is not None:
                emit_out(*pending_out)
                pending_out = None

            # P2: sqrt (sqrt table set), in place
            for cg in range(NCG):
                nc.scalar.sqrt(r2[cg][:], r2[cg][:])

            # P3: sigmoid + arctan (sigmoid_and_others set) + theta cumsum + range-reduce
            mt = [None] * NCG; thr = [None] * NCG
            for cg in range(NCG):
                m = scr.tile([P, T], fp32, tag="scr")
                nc.scalar.activation(m[:], r2[cg][:], AF.Sigmoid)
                mt[cg] = cpool.tile([P, T], fp32, tag=f"mt{cg}", name=f"mt{cg}_{blk}")
                nc.vector.tensor_tensor(mt[cg][:], m[:], sgn[cg][:], OP.mult)
                th0 = scr.tile([P, T], fp32, tag="scr")
                nc.scalar.activation(th0[:], ratio[cg][:], AF.Arctan)
                th = scr.tile([P, T], fp32, tag="scr")
                nc.vector.tensor_tensor(th[:], th0[:], sgn[cg][:], OP.mult)
                Th = kpool.tile([P, T], fp32, tag=f"Th{cg}")
                init = 0.0 if blk == 0 else prevThC[cg][:]
                nc.vector.tensor_tensor_scan(Th[:], th[:], th[:], init,
                                             OP.add, OP.bypass)
                ThC = carry.tile([P, 1], fp32, tag=f"ThC{cg}", name=f"ThC{cg}_{blk}")
                nc.vector.tensor_copy(ThC[:], Th[:, T - 1:T])
                prevThC[cg] = ThC
                kro = scr.tile([P, T], fp32, tag="scr")
                nc.gpsimd.tensor_scalar(kro[:], Th[:], INV2PI, None, OP.mult)
                kr2 = scr.tile([P, T], fp32, tag="scr")
                nc.gpsimd.tensor_scalar(kr2[:], kro[:], MAGIC, MAGIC, OP.add, OP.subtract)
                thr[cg] = cpool.tile([P, T], fp32, tag=f"thr{cg}", name=f"thr{cg}_{blk}")
                nc.vector.cody_waite_cascade(thr[cg][:], Th[:], kr2[:], C1, C2, C3)

            # P4: sin/cos + silu (silu_and_others set) + scans + output assembly
            ys = [None] * NCG
            for cg in range(NCG):
                ui = cpool.tile([P, T], bf, tag=f"ui{cg}", name=f"ui{cg}_{blk}")
                nc.scalar.activation(ui[:], thr[cg][:], AF.Sin)
                thc = scr.tile([P, T], fp32, tag="scr")
                nc.vector.add_range_wrap(thc[:], thr[cg][:], PIH, PI,
                                         float(np.float32(TWO_PI)))
                ur = cpool.tile([P, T], fp32, tag=f"ur{cg}", name=f"ur{cg}_{blk}")
                nc.scalar.activation(ur[:], thc[:], AF.Sin)
                sg = scr.tile([P, T], fp32, tag="scr")
                nc.scalar.activation(sg[:], gs[cg][:], AF.Silu)
                p = scr.tile([P, T], fp32, tag="scr")
                nc.vector.tensor_tensor(p[:], qs[cg][:], sg[:], OP.mult)
                wr = scr.tile([P, T], fp32, tag="scr")
                nc.gpsimd.tensor_tensor(wr[:], kv[cg][:], ur[:], OP.mult)
                wi = scr.tile([P, T], fp32, tag="scr")
                nc.vector.tensor_tensor(wi[:], kv[cg][:], ui[:], OP.mult)
                Zr = kpool.tile([P, T], fp32, tag=f"Zr{cg}")
                initr = 0.0 if blk == 0 else prevZrC[cg][:]
                nc.vector.tensor_tensor_scan(Zr[:], mt[cg][:], wr[:], initr,
                                             OP.mult, OP.add)
                ZrC = carry.tile([P, 1], fp32, tag=f"ZrC{cg}", name=f"ZrC{cg}_{blk}")
                nc.vector.tensor_copy(ZrC[:], Zr[:, T - 1:T])
                prevZrC[cg] = ZrC
                Zi = kpool.tile([P, T], fp32, tag=f"Zi{cg}")
                initi = 0.0 if blk == 0 else prevZiC[cg][:]
                nc.vector.tensor_tensor_scan(Zi[:], mt[cg][:], wi[:], initi,
                                             OP.mult, OP.add)
                ZiC = carry.tile([P, 1], fp32, tag=f"ZiC{cg}", name=f"ZiC{cg}_{blk}")
                nc.vector.tensor_copy(ZiC[:], Zi[:, T - 1:T])
                prevZiC[cg] = ZiC
                t1 = scr.tile([P, T], fp32, tag="scr")
                nc.gpsimd.tensor_tensor(t1[:], ur[:], Zr[:], OP.mult)
                t2 = scr.tile([P, T], fp32, tag="scr")
                nc.vector.tensor_tensor(t2[:], ui[:], Zi[:], OP.mult)
                re = scr.tile([P, T], fp32, tag="scr")
                nc.vector.tensor_tensor(re[:], t1[:], t2[:], OP.add)
                ys[cg] = ypool.tile([P, T], bf, tag=f"y{cg}", name=f"y{cg}_{blk}")
                nc.vector.tensor_tensor(ys[cg][:], re[:], p[:], OP.mult)

            pending_out = (blk, ys)

        emit_out(*pending_out)

    nc.finalize()
    return nc


def _get_nc():
    global _NC
    if _NC is None:
        _NC = _build()
    return _NC


def kernel(**inputs):
    global LAST_RESULT
    from concourse.bass_utils import run_bass_kernel_spmd

    x = np.asarray(inputs["x"], np.float32)
    gamma = np.asarray(inputs["gamma"], np.float32)
    wq = np.asarray(inputs["wq"], np.float32)
    wk = np.asarray(inputs["wk"], np.float32)
    wv = np.asarray(inputs["wv"], np.float32)
    wa = np.asarray(inputs["wa"], np.float32)
    wg = np.asarray(inputs["wg"], np.float32)
    wo = np.asarray(inputs["wo"], np.float32)

    inv = 1.0 / np.sqrt((x * x).sum(-1, keepdims=True) + np.float32(EPS))
    xn = (inv * x * gamma * np.float32(math.sqrt(D))).astype(np.float32)
    xnT = np.ascontiguousarray(xn.transpose(0, 2, 1)).astype(BF16)  # (B, D, N)

    in_maps = []
    for core in range(8):
        b, h = core // 2, core % 2
        ch = slice(h * CH, (h + 1) * CH)
        in_maps.append({
            "xnT": xnT[b],
            "wq": np.ascontiguousarray(wq[:, ch]).astype(BF16),
            "wk": np.ascontiguousarray(wk[:, ch]).astype(BF16),
            "wv": np.ascontiguousarray(wv[:, ch]).astype(BF16),
            "wg": np.ascontiguousarray(wg[:, ch]).astype(BF16),
            "war": np.ascontiguousarray(wa[:, h * CH:(h + 1) * CH]).astype(BF16),
            "wai": np.ascontiguousarray(wa[:, D + h * CH:D + (h + 1) * CH]).astype(BF16),
            "wo": np.ascontiguousarray(wo[ch, :]).astype(BF16),
        })

    nc = _get_nc()
    trace = bool(int(os.environ.get("GATELOOP_TRACE", "0")))
    LAST_RESULT = run_bass_kernel_spmd(
        nc, in_maps, core_ids=list(range(8)), trace=trace,
        trace_cores=list(range(8)) if trace else None,
    )
    res = LAST_RESULT.results

    out = np.empty((B, N, D), np.float32)
    for b in range(B):
        acc = res[2 * b]["outT"] + res[2 * b + 1]["outT"]   # (D, N)
        out[b] = acc.T
    return out


# revision 6
# speedup vs baseline: 1.6344x; 1.6344x over previous
"""GateLoop (B=4, N=4096, D=1024) Trainium2 kernel over 8 NeuronCores.

Sharding: data-parallel over the 4 batch elements x 2-way tensor-parallel
split of the D=1024 recurrence channels (the complex diagonal recurrence is
independent per channel). Core c handles batch c//2, channels
[(c%2)*512 : (c%2+1)*512]. Each core computes its projections, runs the
scan over the full sequence for its 512 channels, and produces a partial
y @ wo[ch, :] of shape (1024, 4096) (transposed). The host sums the two
partials per batch and transposes back. No cross-core communication.

Scan formulation (avoids complex arithmetic + overflow): with
a_t = m_t * cis(phi_t), m_t = sigmoid(|a_t|), write theta_t =
arctan(ai/ar) in (-pi/2, pi/2) and fold the ar<0 half-plane flip into a
SIGNED real multiplier mt_t = m_t * sign(ar_t). With U_t = cis(Theta_t),
Theta_t = cumsum(theta), and S_t = U_t * Z_t the recurrence
S_t = a_t S_{t-1} + kv_t becomes two independent REAL first-order scans
    Zr_t = mt_t * Zr_{t-1} + kv_t * cos(Theta_t)
    Zi_t = mt_t * Zi_{t-1} + kv_t * sin(Theta_t)
and Re(S_t) = cos(Theta_t) * Zr_t + sin(Theta_t) * Zi_t, which map onto
the DVE TensorTensorScan instruction (fp32 state, |mt| < 1 so stable).

v3 performance structure:
 - The output projection of block b is emitted AFTER the projections of
   block b+1 so the PE never waits on the elementwise chain: it streams
   proj(b+1) then out(b) back-to-back and stays at the 2.4 GHz p-state.
 - war/wai PSUM banks are evacuated with plain copies first, derived math
   reads the SBUF copies (banks freed in ~0.7us instead of ~2.5us).
 - Elementwise work is split: GpSimd takes off-critical-path fp32 ops
   (squares, r2, ratio, wr, t1, magic-rounding); DVE keeps the scan
   backbone, PSUM-reading ops, and the short output tail; ACT keeps the
   table functions grouped (sqrt | sigmoid+arctan | sin+silu) to bound
   activation-table loads.
"""
import math
import os

import numpy as np
import ml_dtypes

B, N, D = 4, 4096, 1024
CH = 512            # channels per core (tensor-parallel half)
NCG = CH // 128     # 4 channel groups of 128 partitions
T = 512             # token block
NBLK = N // T
P = 128
KT = D // P         # contraction tiles
EPS = 1e-5
BF16 = ml_dtypes.bfloat16

TWO_PI = 2 * math.pi
C1 = float(np.float32(6.28125))
C2 = float(np.float32(np.float64(TWO_PI) - 6.28125))
C3 = float(np.float32(np.float64(TWO_PI) - 6.28125
                      - np.float64(np.float32(np.float64(TWO_PI) - 6.28125))))
MAGIC = float(np.float32(1.5 * 2 ** 23))
INV2PI = float(np.float32(1.0 / TWO_PI))
PI = float(np.float32(math.pi))
PIH = float(np.float32(math.pi / 2))

_NC = None
LAST_RESULT = None  # BassKernelResults of the most recent run (for profiling)


def _build():
    from contextlib import ExitStack
    from concourse import bacc
    import concourse.mybir as mybir
    import concourse.tile as tile
    from concourse.mybir import ActivationFunctionType as AF, AluOpType as OP

    fp32 = mybir.dt.float32
    bf = mybir.dt.bfloat16

    nc = bacc.Bacc(None, target_bir_lowering=False)

    xnT_d = nc.dram_tensor("xnT", [D, N], bf, kind="ExternalInput")
    wnames = ["wq", "wk", "wv", "wg", "war", "wai"]
    w_d = {n: nc.dram_tensor(n, [D, CH], bf, kind="ExternalInput") for n in wnames}
    wo_d = nc.dram_tensor("wo", [CH, D], bf, kind="ExternalInput")
    outT_d = nc.dram_tensor("outT", [D, N], fp32, kind="ExternalOutput")

    xnT_t = xnT_d.rearrange("(ko p) n -> p ko n", p=P)
    outT_t = outT_d.rearrange("(mo p) n -> p mo n", p=P)

    with tile.TileContext(nc) as tc, ExitStack() as ctx:
        wpool = ctx.enter_context(tc.tile_pool(name="w", bufs=1))
        xpool = ctx.enter_context(tc.tile_pool(name="x", bufs=2))
        cpool = ctx.enter_context(tc.tile_pool(name="c", bufs=1))   # per-cg, within-block
        kpool = ctx.enter_context(tc.tile_pool(name="k", bufs=1))   # scan outputs (carry via *_C)
        carry = ctx.enter_context(tc.tile_pool(name="cc", bufs=2))  # [P,1] scan carries
        scr = ctx.enter_context(tc.tile_pool(name="s", bufs=10))    # short-lived scratch
        obp = ctx.enter_context(tc.tile_pool(name="o", bufs=2))
        ypool = ctx.enter_context(tc.tile_pool(name="y", bufs=2))
        pproj = ctx.enter_context(tc.tile_pool(name="pp", bufs=6, space="PSUM"))
        pout = ctx.enter_context(tc.tile_pool(name="po", bufs=2, space="PSUM"))

        wsb = {}
        for n in wnames:
            t_ = wpool.tile([P, KT, CH], bf, tag=f"w_{n}")
            nc.sync.dma_start(t_[:], w_d[n].rearrange("(ko p) m -> p ko m", p=P))
            wsb[n] = t_
        wosb = wpool.tile([P, CH // P, D], bf, tag="w_wo")
        nc.sync.dma_start(wosb[:], wo_d.rearrange("(ko p) m -> p ko m", p=P))

        prevThC = [None] * NCG
        prevZrC = [None] * NCG
        prevZiC = [None] * NCG
        pending_out = None  # (blk, ys[4]) whose output matmul is deferred

        def emit_out(blk, ys):
            t0 = blk * T
            for mo in range(D // P):
                pso = pout.tile([P, T], fp32, tag="out")
                for cg in range(NCG):
                    nc.tensor.matmul(pso[:], wosb[:, cg, mo * P:(mo + 1) * P],
                                     ys[cg][:], start=(cg == 0), stop=(cg == NCG - 1))
                ob = obp.tile([P, T], fp32, tag="ob")
                nc.scalar.copy(ob[:], pso[:])
                nc.sync.dma_start(outT_t[:, mo, t0:t0 + T], ob[:])

        for blk in range(NBLK):
            t0 = blk * T
            xb = xpool.tile([P, KT, T], bf, tag="xb")
            nc.sync.dma_start(xb[:], xnT_t[:, :, t0:t0 + T])

            kv = [None] * NCG; qs = [None] * NCG; gs = [None] * NCG
            sgn = [None] * NCG; ratio = [None] * NCG; r2 = [None] * NCG
            for cg in range(NCG):
                cs = slice(cg * P, (cg + 1) * P)
                PS = {}
                for n in wnames:
                    ps = pproj.tile([P, T], fp32, tag="proj")
                    for k in range(KT):
                        nc.tensor.matmul(ps[:], wsb[n][:, k, cs], xb[:, k, :],
                                         start=(k == 0), stop=(k == KT - 1))
                    PS[n] = ps
                    # evacuate each PSUM bank as soon as its matmuls finish
                    if n == "wq":
                        qs[cg] = cpool.tile([P, T], bf, tag=f"qs{cg}", name=f"qs{cg}_{blk}")
                        nc.scalar.copy(qs[cg][:], ps[:])
                    elif n == "wk":
                        pass  # read by kv right after wv's copy lands
                    elif n == "wv":
                        vs = scr.tile([P, T], fp32, tag="scr", name=f"vs{cg}_{blk}")
                        nc.scalar.copy(vs[:], ps[:])
                        kv[cg] = cpool.tile([P, T], fp32, tag=f"kv{cg}", name=f"kv{cg}_{blk}")
                        nc.vector.tensor_tensor(kv[cg][:], PS["wk"][:], vs[:], OP.mult)
                    elif n == "wg":
                        gs[cg] = cpool.tile([P, T], bf, tag=f"gs{cg}", name=f"gs{cg}_{blk}")
                        nc.scalar.copy(gs[cg][:], ps[:])
                    elif n == "war":
                        war_s = scr.tile([P, T], fp32, tag="scr", name=f"war{cg}_{blk}")
                        nc.scalar.copy(war_s[:], ps[:])
                        PS["war_s"] = war_s
                    elif n == "wai":
                        wai_s = scr.tile([P, T], fp32, tag="scr", name=f"wai{cg}_{blk}")
                        nc.scalar.copy(wai_s[:], ps[:])
                        PS["wai_s"] = wai_s
                # derived P1 math from the SBUF copies
                war_s, wai_s = PS["war_s"], PS["wai_s"]
                sq = scr.tile([P, T], fp32, tag="scr")
                nc.scalar.square(sq[:], war_s[:])
                sq2 = scr.tile([P, T], fp32, tag="scr")
                nc.scalar.square(sq2[:], wai_s[:])
                r2[cg] = cpool.tile([P, T], fp32, tag=f"r2{cg}", name=f"r2{cg}_{blk}")
                nc.gpsimd.tensor_tensor(r2[cg][:], sq[:], sq2[:], OP.add)
                sgn[cg] = cpool.tile([P, T], bf, tag=f"sgn{cg}", name=f"sgn{cg}_{blk}")
                nc.scalar.sign(sgn[cg][:], war_s[:])
                aabs = scr.tile([P, T], fp32, tag="scr")
                nc.scalar.activation(aabs[:], war_s[:], AF.Abs)
                nc.vector.tensor_scalar(aabs[:], aabs[:], 1e-4, None, OP.max)
                rec = scr.tile([P, T], fp32, tag="scr")
                nc.vector.reciprocal_approx_fast(rec[:], aabs[:])
                ratio[cg] = cpool.tile([P, T], fp32, tag=f"ratio{cg}", name=f"ratio{cg}_{blk}")
                nc.gpsimd.tensor_tensor(ratio[cg][:], wai_s[:], rec[:], OP.mult)

            # deferred output projection of the PREVIOUS block: the PE flows
            # straight from this block's projections into it, no stall.
            if pending_out is not None:
                emit_out(*pending_out)
                pending_out = None

            # P2: sqrt (sqrt table set), in place
            for cg in range(NCG):
                nc.scalar.sqrt(r2[cg][:], r2[cg][:])

            # P3: sigmoid + arctan (sigmoid_and_others set) + theta cumsum + range-reduce
            mt = [None] * NCG; thr = [None] * NCG
            for cg in range(NCG):
                m = scr.tile([P, T], fp32, tag="scr")
                nc.scalar.activation(m[:], r2[cg][:], AF.Sigmoid)
                mt[cg] = cpool.tile([P, T], fp32, tag=f"mt{cg}", name=f"mt{cg}_{blk}")
                nc.vector.tensor_tensor(mt[cg][:], m[:], sgn[cg][:], OP.mult)
                th0 = scr.tile([P, T], fp32, tag="scr")
                nc.scalar.activation(th0[:], ratio[cg][:], AF.Arctan)
                th = scr.tile([P, T], fp32, tag="scr")
                nc.vector.tensor_tensor(th[:], th0[:], sgn[cg][:], OP.mult)
                Th = kpool.tile([P, T], fp32, tag=f"Th{cg}")
                init = 0.0 if blk == 0 else prevThC[cg][:]
                nc.vector.tensor_tensor_scan(Th[:], th[:], th[:], init,
                                             OP.add, OP.bypass)
                ThC = carry.tile([P, 1], fp32, tag=f"ThC{cg}", name=f"ThC{cg}_{blk}")
                nc.vector.tensor_copy(ThC[:], Th[:, T - 1:T])
                prevThC[cg] = ThC
                kro = scr.tile([P, T], fp32, tag="scr")
                nc.vector.tensor_scalar(kro[:], Th[:], INV2PI, None, OP.mult)
                kr2 = scr.tile([P, T], fp32, tag="scr")
                nc.vector.tensor_scalar(kr2[:], kro[:], MAGIC, MAGIC, OP.add, OP.subtract)
                thr[cg] = cpool.tile([P, T], fp32, tag=f"thr{cg}", name=f"thr{cg}_{blk}")
                nc.vector.cody_waite_cascade(thr[cg][:], Th[:], kr2[:], C1, C2, C3)

            # P4: sin/cos + silu (silu_and_others set) + scans + output assembly
            ys = [None] * NCG
            for cg in range(NCG):
                ui = cpool.tile([P, T], bf, tag=f"ui{cg}", name=f"ui{cg}_{blk}")
                nc.scalar.activation(ui[:], thr[cg][:], AF.Sin)
                thc = scr.tile([P, T], fp32, tag="scr")
                nc.vector.add_range_wrap(thc[:], thr[cg][:], PIH, PI,
                                         float(np.float32(TWO_PI)))
                ur = cpool.tile([P, T], fp32, tag=f"ur{cg}", name=f"ur{cg}_{blk}")
                nc.scalar.activation(ur[:], thc[:], AF.Sin)
                sg = scr.tile([P, T], fp32, tag="scr")
                nc.scalar.activation(sg[:], gs[cg][:], AF.Silu)
                p = scr.tile([P, T], fp32, tag="scr")
                nc.vector.tensor_tensor(p[:], qs[cg][:], sg[:], OP.mult)
                wr = scr.tile([P, T], fp32, tag="scr")
                nc.vector.tensor_tensor(wr[:], kv[cg][:], ur[:], OP.mult)
                wi = scr.tile([P, T], fp32, tag="scr")
                nc.vector.tensor_tensor(wi[:], kv[cg][:], ui[:], OP.mult)
                Zr = kpool.tile([P, T], fp32, tag=f"Zr{cg}")
                initr = 0.0 if blk == 0 else prevZrC[cg][:]
                nc.vector.tensor_tensor_scan(Zr[:], mt[cg][:], wr[:], initr,
                                             OP.mult, OP.add)
                ZrC = carry.tile([P, 1], fp32, tag=f"ZrC{cg}", name=f"ZrC{cg}_{blk}")
                nc.vector.tensor_copy(ZrC[:], Zr[:, T - 1:T])
                prevZrC[cg] = ZrC
                Zi = kpool.tile([P, T], fp32, tag=f"Zi{cg}")
                initi = 0.0 if blk == 0 else prevZiC[cg][:]
                nc.vector.tensor_tensor_scan(Zi[:], mt[cg][:], wi[:], initi,
                                             OP.mult, OP.add)
                ZiC = carry.tile([P, 1], fp32, tag=f"ZiC{cg}", name=f"ZiC{cg}_{blk}")
                nc.vector.tensor_copy(ZiC[:], Zi[:, T - 1:T])
                prevZiC[cg] = ZiC
                t1 = scr.tile([P, T], fp32, tag="scr")
                nc.gpsimd.tensor_tensor(t1[:], ur[:], Zr[:], OP.mult)
                t2 = scr.tile([P, T], fp32, tag="scr")
                nc.vector.tensor_tensor(t2[:], ui[:], Zi[:], OP.mult)
                re = scr.tile([P, T], fp32, tag="scr")
                nc.vector.tensor_tensor(re[:], t1[:], t2[:], OP.add)
                ys[cg] = ypool.tile([P, T], bf, tag=f"y{cg}", name=f"y{cg}_{blk}")
                nc.vector.tensor_tensor(ys[cg][:], re[:], p[:], OP.mult)

            pending_out = (blk, ys)

        emit_out(*pending_out)

    nc.finalize()
    return nc


def _get_nc():
    global _NC
    if _NC is None:
        _NC = _build()
    return _NC


def kernel(**inputs):
    global LAST_RESULT
    from concourse.bass_utils import run_bass_kernel_spmd

    x = np.asarray(inputs["x"], np.float32)
    gamma = np.asarray(inputs["gamma"], np.float32)
    wq = np.asarray(inputs["wq"], np.float32)
    wk = np.asarray(inputs["wk"], np.float32)
    wv = np.asarray(inputs["wv"], np.float32)
    wa = np.asarray(inputs["wa"], np.float32)
    wg = np.asarray(inputs["wg"], np.float32)
    wo = np.asarray(inputs["wo"], np.float32)

    inv = 1.0 / np.sqrt((x * x).sum(-1, keepdims=True) + np.float32(EPS))
    xn = (inv * x * gamma * np.float32(math.sqrt(D))).astype(np.float32)
    xnT = np.ascontiguousarray(xn.transpose(0, 2, 1)).astype(BF16)  # (B, D, N)

    in_maps = []
    for core in range(8):
        b, h = core // 2, core % 2
        ch = slice(h * CH, (h + 1) * CH)
        in_maps.append({
            "xnT": xnT[b],
            "wq": np.ascontiguousarray(wq[:, ch]).astype(BF16),
            "wk": np.ascontiguousarray(wk[:, ch]).astype(BF16),
            "wv": np.ascontiguousarray(wv[:, ch]).astype(BF16),
            "wg": np.ascontiguousarray(wg[:, ch]).astype(BF16),
            "war": np.ascontiguousarray(wa[:, h * CH:(h + 1) * CH]).astype(BF16),
            "wai": np.ascontiguousarray(wa[:, D + h * CH:D + (h + 1) * CH]).astype(BF16),
            "wo": np.ascontiguousarray(wo[ch, :]).astype(BF16),
        })

    nc = _get_nc()
    trace = bool(int(os.environ.get("GATELOOP_TRACE", "0")))
    LAST_RESULT = run_bass_kernel_spmd(
        nc, in_maps, core_ids=list(range(8)), trace=trace,
        trace_cores=list(range(8)) if trace else None,
    )
    res = LAST_RESULT.results

    out = np.empty((B, N, D), np.float32)
    for b in range(B):
        acc = res[2 * b]["outT"] + res[2 * b + 1]["outT"]   # (D, N)
        out[b] = acc.T
    return out
